# revision 32
# baseline (speedup 1.0000x reference)
"""Trainium2 Bass kernel for degree-3 real spherical-harmonics evaluation.

Computes, for N=2M points with 16 SH coefficients x 2 channels each:
    d    = normalize(coordinates - rx_pos)
    out  = sum_k basis_k(d) * sh[n, k, c]

Strategy (8 NeuronCores, data-parallel over points):
  - Pad N to 2,007,040 = 8 cores * 128 partitions * 1960 points and give each
    core a contiguous slab. Per core, point n lives at (partition p = n//1960,
    f = n%1960); all DMAs are large and fully contiguous per partition.
  - sh coefficients are DMA'd with an inline fp32->bf16 cast (SWDGE), then one
    ScalarE copy de-interleaves them into per-(k,c) planes so the vector MAC
    runs in bf16 2x perf mode with unit stride.
  - The SH basis is built from C1-scaled unit vectors; every SH constant is
    folded into fused DVE ops (scalar_tensor_tensor / affine_mul_reduce /
    tensor_scalar), so no separate scale passes are needed.
  - MAC: 15 broadcasted bf16 multiplies (both channels per instruction) and a
    16-term binary add tree, all in 2x mode.
"""

import ml_dtypes
import numpy as np

import concourse.bass as bass
import concourse.tile as tile
from concourse import bacc, mybir
from concourse.bass_utils import run_bass_kernel_spmd

f32 = mybir.dt.float32
bf16 = mybir.dt.bfloat16
AF = mybir.ActivationFunctionType
OP = mybir.AluOpType

# ----- problem constants (hardcoded per spec) -----
N = 2_000_000
K = 16
CH = 2
ACTIVE_DEG = 3

C0 = 0.28209479177387814
C1 = 0.4886025119029199
C2 = (1.0925484305920792, -1.0925484305920792, 0.31539156525252005,
      -1.0925484305920792, 0.5462742152960396)
C3 = (-0.5900435899266435, 2.890611442640554, -0.4570457994644658,
      0.3731763325901154, -0.4570457994644658, 1.445305721320277,
      -0.5900435899266435)

# Basis constants with the C1 hat-scaling folded in (hats carry a factor C1).
_C12 = C1 * C1
_C13 = C1 * C1 * C1
CC4 = C2[0] / _C12
CC5 = C2[1] / _C12
A6, D6 = 3.0 * C2[2] / _C12, -C2[2]
CC7 = C2[3] / _C12
CC8 = C2[4] / _C12
CC9 = C3[0] / _C13
CC10 = C3[1] / _C13
A11, D11 = 5.0 * C3[2] / _C13, -C3[2] / C1
A12, D12 = 5.0 * C3[3] / _C13, -3.0 * C3[3] / C1
A13, D13 = 5.0 * C3[4] / _C13, -C3[4] / C1
CC14 = C3[5] / _C13
CC15 = C3[6] / _C13

# ----- sharding geometry -----
NCORES = 8
PPART = 1960                 # points per partition per core
PC = 128 * PPART             # points per core = 250,880
NPAD = NCORES * PC           # 2,007,040
TF = 490                     # points per partition per tile
NT = PPART // TF             # 5 tiles


def _build_nc():
    # Inputs arrive host-preprocessed: coords as 3 planes [3, PC] fp32 and
    # sh as 32 (k,c)-planes [32, PC] bf16 — every DMA lands in the exact
    # SBUF layout compute wants, no on-chip shuffling.
    nc = bacc.Bacc("TRN2")
    coords_ext = nc.declare_dram_parameter("coords", [3, PC], f32, isOutput=False)
    sh_ext = nc.declare_dram_parameter("sh", [32, PC], bf16, isOutput=False)
    consts_ext = nc.declare_dram_parameter("consts", [128, 8], f32, isOutput=False)
    out_ext = nc.declare_dram_parameter("out", [PC, CH], f32, isOutput=True)

    coords_ap = coords_ext[:].rearrange("c (p f) -> p c f", p=128)   # [128,3,1960]
    sh_ap = sh_ext[:].rearrange("j (p f) -> p j f", p=128)           # [128,32,1960]
    out_ap = out_ext[:].rearrange("(p f) c -> p (f c)", p=128)       # [128, 3920]

    F = TF
    with tile.TileContext(nc) as tc:
        with (
            tc.tile_pool(name="pconst", bufs=1) as pconst,
            tc.tile_pool(name="psh", bufs=2) as psh,
            tc.tile_pool(name="pco", bufs=2) as pco,
            tc.tile_pool(name="psq", bufs=2) as psq,
            tc.tile_pool(name="pr", bufs=2) as pr,
            tc.tile_pool(name="ph", bufs=2) as ph,
            tc.tile_pool(name="pmono", bufs=2) as pmono,
            tc.tile_pool(name="pb", bufs=3) as pb,
            tc.tile_pool(name="pm", bufs=3) as pm,
            tc.tile_pool(name="ptree", bufs=4) as ptree,
            tc.tile_pool(name="pacc", bufs=2) as pacc,
            tc.tile_pool(name="pout", bufs=2) as pout,
            tc.tile_pool(name="pscr", bufs=2) as pscr,
        ):
            ct = pconst.tile([128, 8], f32)
            nc.sync.dma_start(out=ct[:], in_=consts_ext[:])

            for t in range(NT):
                shin = psh.tile([128, F * 32], bf16, tag="shin")
                shin3 = shin[:].rearrange("p (j f) -> p j f", f=F)
                nc.sync.dma_start(
                    out=shin3[:, 0:16, :],
                    in_=sh_ap[:, 0:16, t * F:(t + 1) * F],
                )
                nc.sync.dma_start(
                    out=shin3[:, 16:32, :],
                    in_=sh_ap[:, 16:32, t * F:(t + 1) * F],
                )
                ctile = pco.tile([128, F * 3], f32, tag="ctile")
                nc.gpsimd.dma_start(
                    out=ctile[:].rearrange("p (c f) -> p c f", f=F),
                    in_=coords_ap[:, :, t * F:(t + 1) * F],
                )

                cv = ctile[:].rearrange("p (c f) -> p c f", c=3)  # plane comps

                # d = coords - rx is precomputed on the host; square all
                # three planes in one ScalarE op
                sq = psq.tile([128, 3 * F], f32, tag="sq", bufs=1)
                nc.scalar.activation(sq[:], ctile[:], AF.Square, bias=0.0,
                                     scale=1.0)

                r2a = pr.tile([128, F], f32, tag="r2a")
                nc.vector.tensor_add(r2a[:], sq[:, 0:F], sq[:, F:2 * F])
                nc.vector.scalar_tensor_tensor(
                    r2a[:], sq[:, 2 * F:3 * F], 1e-12, r2a[:], OP.add, OP.add
                )
                inv = pr.tile([128, F], f32, tag="inv")
                nc.vector.reciprocal_approx_fast(inv[:], r2a[:])
                # sqrt(C1^2 / r2) = C1 * rsqrt(r2)
                rinv = inv
                nc.scalar.activation(rinv[:], inv[:], AF.Sqrt, bias=0.0,
                                     scale=_C12)

                # C1-scaled unit vector: d * rinvC1, rinv broadcast over the
                # three component planes in a single VectorE op
                hats = ph.tile([128, 3 * F], f32, tag="hats")
                nc.vector.tensor_tensor(
                    hats[:].rearrange("p (c f) -> p c f", c=3),
                    ctile[:].rearrange("p (c f) -> p c f", c=3),
                    rinv[:].unsqueeze(1).broadcast_to((128, 3, F)),
                    OP.mult,
                )
                X = hats[:, 0:F]
                Y = hats[:, F:2 * F]
                Z = hats[:, 2 * F:3 * F]

                sqh = ph.tile([128, 3 * F], f32, tag="sqh", bufs=1)
                nc.scalar.activation(sqh[:], hats[:], AF.Square, bias=0.0,
                                     scale=1.0)
                XX = sqh[:, 0:F]
                YY = sqh[:, F:2 * F]
                ZZ = sqh[:, 2 * F:3 * F]

                xy = pmono.tile([128, F], f32, tag="xy")
                nc.vector.tensor_mul(xy[:], X, Y)
                t8 = pmono.tile([128, F], f32, tag="t8")
                nc.vector.tensor_sub(t8[:], XX, YY)
                u9 = pmono.tile([128, F], f32, tag="u9")
                nc.vector.affine_then_add(u9[:], XX, t8[:], 2.0, 0.0)
                u15 = pmono.tile([128, F], f32, tag="u15")
                nc.vector.affine_then_add(u15[:], YY, t8[:], -2.0, 0.0)

                # ---- MAC: pair-batched products ([k2, c, f] = 4F per op) ----
                # Basis planes are packed in pairs matching consecutive k so
                # each product instruction covers 2 k's x 2 channels, and the
                # add tree runs on [4F] chunks (terms halve each level).
                def bpair_tile():
                    return pb.tile([128, 2 * F], bf16, tag="b", name="bp")

                def mk_product2(p_idx, bp):
                    m = pm.tile([128, 4 * F], bf16, tag="m", name="m")
                    in1 = shin[:, 4 * p_idx * F:(4 * p_idx + 4) * F].rearrange(
                        "p (k c f) -> p k c f", k=2, c=2)
                    in0 = bp[:].rearrange("p (k f) -> p k f", k=2) \
                        .unsqueeze(2).broadcast_to((128, 2, 2, F))
                    nc.vector.tensor_tensor(
                        m[:].rearrange("p (k c f) -> p k c f", k=2, c=2),
                        in0, in1, OP.mult)
                    return m

                def amr(out_slice, in0, in1, scale, bias):
                    scr = pscr.tile([128, 1], f32, tag="scr", name="scr")
                    nc.vector.affine_mul_reduce(out_slice, scr[:], in0, in1,
                                                scale, bias)

                def tadd(a, b_, dt):
                    tg = "treeb" if dt == bf16 else "treef"
                    nb = 5 if dt == bf16 else 2
                    o = ptree.tile([128, a.shape[1]], dt, tag=tg, name="tr",
                                   bufs=nb)
                    nc.vector.tensor_tensor(o[:], a[:], b_[:], OP.add)
                    return o

                # hb holds [C0, -Ytilde, +Ztilde, -Xtilde, +Ztilde, -Ytilde]:
                # slots 0-3 are the first two basis pairs; [2F:4F] doubles as
                # the (Z,X) hat pair and [4F:6F] as the (Z,Y) hat pair for the
                # paired deg>=2 plane multiplies.
                hb = pb.tile([128, 6 * F], bf16, tag="hb", name="hb", bufs=2)
                nc.vector.memset(hb[:, 0:F], C0)
                nc.scalar.mul(hb[:, F:2 * F], Y, -1.0)
                nc.scalar.copy(hb[:, 2 * F:3 * F], Z)
                nc.scalar.mul(hb[:, 3 * F:4 * F], X, -1.0)
                nc.scalar.copy(hb[:, 4 * F:5 * F], Z)
                nc.scalar.mul(hb[:, 5 * F:6 * F], Y, -1.0)
                Yn = hb[:, F:2 * F]
                Zb = hb[:, 2 * F:3 * F]
                ZXn = hb[:, 2 * F:4 * F]
                ZYn = hb[:, 4 * F:6 * F]
                m0 = mk_product2(0, hb[:, 0:2 * F])
                m1 = mk_product2(1, hb[:, 2 * F:4 * F])
                A = tadd(m0, m1, bf16)

                # ScalarE pre-scales one factor of each remaining plane to
                # bf16 (constants and signs folded), VectorE finishes with a
                # bf16 2x multiply against a signed hat (pair) plane.
                def spre(out_slice, pre_in, scale, bias=None):
                    if bias is None:
                        nc.scalar.mul(out_slice, pre_in, scale)
                    else:
                        nc.scalar.activation(out_slice, pre_in, AF.Identity,
                                             bias=bias, scale=scale)

                def sb_tile(w):
                    return pb.tile([128, w * F], bf16, tag="sb", name="sb",
                                   bufs=4)

                # pair (k4, k5): [c4*xy, c5*Y*Z]
                bp2 = bpair_tile()
                nc.scalar.mul(bp2[:, 0:F], xy[:], CC4)
                s5 = sb_tile(1)
                spre(s5[:], Y, CC5)
                nc.vector.tensor_mul(bp2[:, F:2 * F], s5[:], Zb)
                m2 = mk_product2(2, bp2)
                # pair (k6, k7): [a6*ZZ+d6, c7*X*Z]
                bp3 = bpair_tile()
                nc.scalar.activation(bp3[:, 0:F], ZZ, AF.Identity,
                                     bias=ct[:, 3:4], scale=A6)
                s7 = sb_tile(1)
                spre(s7[:], X, CC7)
                nc.vector.tensor_mul(bp3[:, F:2 * F], s7[:], Zb)
                m3 = mk_product2(3, bp3)
                B = tadd(m2, m3, bf16)

                # pair (k8, k9): [c8*t8, c9*u9*Y]
                bp4 = bpair_tile()
                nc.scalar.mul(bp4[:, 0:F], t8[:], CC8)
                s9 = sb_tile(1)
                spre(s9[:], u9[:], -CC9)
                nc.vector.tensor_mul(bp4[:, F:2 * F], s9[:], Yn)
                m4 = mk_product2(4, bp4)
                # pair (k10, k11): [c10*xy*Z, (a11*ZZ+d11)*Y] via (Z,Y) pair
                sp5 = sb_tile(2)
                spre(sp5[:, 0:F], xy[:], CC10)
                spre(sp5[:, F:2 * F], ZZ, -A11, bias=ct[:, 4:5])
                bp5 = bpair_tile()
                nc.vector.tensor_mul(bp5[:], sp5[:], ZYn)
                m5 = mk_product2(5, bp5)
                Cc = tadd(m4, m5, bf16)

                # pair (k12, k13): [(a12*ZZ+d12)*Z, (a13*ZZ+d13)*X] via (Z,X)
                sp6 = sb_tile(2)
                spre(sp6[:, 0:F], ZZ, A12, bias=ct[:, 5:6])
                spre(sp6[:, F:2 * F], ZZ, -A13, bias=ct[:, 6:7])
                bp6 = bpair_tile()
                nc.vector.tensor_mul(bp6[:], sp6[:], ZXn)
                m6 = mk_product2(6, bp6)
                # pair (k14, k15): [c14*t8*Z, c15*u15*X] via (Z,X) pair
                sp7 = sb_tile(2)
                spre(sp7[:, 0:F], t8[:], CC14)
                spre(sp7[:, F:2 * F], u15[:], -CC15)
                bp7 = bpair_tile()
                nc.vector.tensor_mul(bp7[:], sp7[:], ZXn)
                m7 = mk_product2(7, bp7)
                D = tadd(m6, m7, bf16)

                E = tadd(A, B, bf16)
                G = tadd(Cc, D, bf16)
                H = tadd(E, G, bf16)
                acc = pacc.tile([128, 2 * F], f32, tag="acc")
                nc.vector.tensor_add(acc[:], H[:, 0:2 * F], H[:, 2 * F:4 * F])

                # interleave back to (f, c) and upcast to fp32
                out_t = pout.tile([128, 2 * F], f32, tag="out")
                nc.scalar.copy(
                    out_t[:].rearrange("p (f c) -> p c f", c=2),
                    acc[:].rearrange("p (c f) -> p c f", c=2),
                )
                nc.gpsimd.dma_start(
                    out=out_ap[:, t * 2 * F:(t + 1) * 2 * F], in_=out_t[:]
                )

    nc.finalize()
    return nc


_NC_CACHE = None
_last_in_maps = None


def _get_nc():
    global _NC_CACHE
    if _NC_CACHE is None:
        _NC_CACHE = _build_nc()
    return _NC_CACHE


def kernel(coordinates, active_deg, max_coeffs, sh_coefficients, rx_pos,
           **unused):
    assert int(active_deg) == ACTIVE_DEG and int(max_coeffs) == K
    coords = np.ascontiguousarray(np.asarray(coordinates, dtype=np.float32))
    sh = np.ascontiguousarray(np.asarray(sh_coefficients, dtype=np.float32))
    rx = np.asarray(rx_pos, dtype=np.float32).reshape(3)
    n = coords.shape[0]
    assert n == N and sh.shape == (N * K, CH)

    consts = np.zeros((128, 8), dtype=np.float32)
    consts[:, 0:3] = -rx[None, :]
    consts[:, 3] = D6
    consts[:, 4] = -D11
    consts[:, 5] = D12
    consts[:, 6] = -D13

    # Host-side relayout: coords -> 3 fp32 planes, sh -> 32 bf16 (k,c)-planes,
    # so device DMAs land directly in compute layout.
    sh32 = sh.reshape(n, K * CH)
    coordsT = coords.T - rx[:, None]  # [3, N], receiver offset folded in
    in_maps = []
    for c in range(NCORES):
        lo, hi = c * PC, (c + 1) * PC
        real = min(hi, n) - lo
        coords_c = np.zeros((3, PC), dtype=np.float32)
        coords_c[:, :real] = coordsT[:, lo:lo + real]
        sh_c = np.zeros((32, PC), dtype=ml_dtypes.bfloat16)
        sh_c[:, :real] = sh32[lo:lo + real].T
        in_maps.append({"coords": coords_c, "sh": sh_c, "consts": consts})

    global _last_in_maps
    _last_in_maps = in_maps
    res = run_bass_kernel_spmd(_get_nc(), in_maps, list(range(NCORES)))
    out = np.concatenate([np.asarray(res.results[c]["out"])
                          for c in range(NCORES)], axis=0)
    return out[:n]


# revision 33
# speedup vs baseline: 1.0024x; 1.0024x over previous
"""Trainium2 Bass kernel for degree-3 real spherical-harmonics evaluation.

Computes, for N=2M points with 16 SH coefficients x 2 channels each:
    d    = normalize(coordinates - rx_pos)
    out  = sum_k basis_k(d) * sh[n, k, c]

Strategy (8 NeuronCores, data-parallel over points):
  - Pad N to 2,007,040 = 8 cores * 128 partitions * 1960 points; each core
    gets a contiguous slab (point n -> partition n//1960, column n%1960).
  - Host-side relayout inside kernel(): coords become 3 fp32 planes with
    rx_pos pre-subtracted; sh becomes 32 (k,c)-planes cast to bf16. Every
    DMA then lands in the exact SBUF layout compute wants (contiguous
    per-plane runs >= 512B per descriptor) and sh DRAM traffic is halved.
  - Normalization: r^2 via one ScalarE Square + two VectorE adds,
    reciprocal_approx_fast (DVE) + ScalarE Sqrt with the C1 SH constant
    folded into its scale, so the unit vector arrives pre-scaled by C1.
  - Basis: unit-sphere identities collapse deg-2/3 polynomials to affine
    forms of z^2; ScalarE pre-scales one factor of each plane to bf16 with
    all SH constants/signs folded; VectorE finishes with bf16 2x-mode
    multiplies against signed hat planes, packed in pairs so one [2F] op
    builds two basis planes.
  - MAC: 8 rank-4 broadcast tensor_tensor products (2 k's x 2 channels per
    instruction, bf16 2x mode) + a binary add tree on [4F] chunks (bf16
    except the final fp32 fold), then ScalarE re-interleaves/upcasts to the
    fp32 [N,2] output.
"""

import ml_dtypes
import numpy as np

import concourse.bass as bass
import concourse.tile as tile
from concourse import bacc, mybir
from concourse.bass_utils import run_bass_kernel_spmd

f32 = mybir.dt.float32
bf16 = mybir.dt.bfloat16
AF = mybir.ActivationFunctionType
OP = mybir.AluOpType

# ----- problem constants (hardcoded per spec) -----
N = 2_000_000
K = 16
CH = 2
ACTIVE_DEG = 3

C0 = 0.28209479177387814
C1 = 0.4886025119029199
C2 = (1.0925484305920792, -1.0925484305920792, 0.31539156525252005,
      -1.0925484305920792, 0.5462742152960396)
C3 = (-0.5900435899266435, 2.890611442640554, -0.4570457994644658,
      0.3731763325901154, -0.4570457994644658, 1.445305721320277,
      -0.5900435899266435)

# Basis constants with the C1 hat-scaling folded in (hats carry a factor C1).
_C12 = C1 * C1
_C13 = C1 * C1 * C1
CC4 = C2[0] / _C12
CC5 = C2[1] / _C12
A6, D6 = 3.0 * C2[2] / _C12, -C2[2]
CC7 = C2[3] / _C12
CC8 = C2[4] / _C12
CC9 = C3[0] / _C13
CC10 = C3[1] / _C13
A11, D11 = 5.0 * C3[2] / _C13, -C3[2] / C1
A12, D12 = 5.0 * C3[3] / _C13, -3.0 * C3[3] / C1
A13, D13 = 5.0 * C3[4] / _C13, -C3[4] / C1
CC14 = C3[5] / _C13
CC15 = C3[6] / _C13

# ----- sharding geometry -----
NCORES = 8
PPART = 1960                 # points per partition per core
PC = 128 * PPART             # points per core = 250,880
NPAD = NCORES * PC           # 2,007,040
TF = 490                     # points per partition per tile
NT = PPART // TF             # 5 tiles


def _build_nc():
    # Inputs arrive host-preprocessed: coords as 3 planes [3, PC] fp32 and
    # sh as 32 (k,c)-planes [32, PC] bf16 — every DMA lands in the exact
    # SBUF layout compute wants, no on-chip shuffling.
    nc = bacc.Bacc("TRN2")
    coords_ext = nc.declare_dram_parameter("coords", [3, PC], f32, isOutput=False)
    sh_ext = nc.declare_dram_parameter("sh", [32, PC], bf16, isOutput=False)
    consts_ext = nc.declare_dram_parameter("consts", [128, 8], f32, isOutput=False)
    out_ext = nc.declare_dram_parameter("out", [PC, CH], f32, isOutput=True)

    coords_ap = coords_ext[:].rearrange("c (p f) -> p c f", p=128)   # [128,3,1960]
    sh_ap = sh_ext[:].rearrange("j (p f) -> p j f", p=128)           # [128,32,1960]
    out_ap = out_ext[:].rearrange("(p f) c -> p (f c)", p=128)       # [128, 3920]

    F = TF
    with tile.TileContext(nc) as tc:
        with (
            tc.tile_pool(name="pconst", bufs=1) as pconst,
            tc.tile_pool(name="psh", bufs=2) as psh,
            tc.tile_pool(name="pco", bufs=2) as pco,
            tc.tile_pool(name="psq", bufs=2) as psq,
            tc.tile_pool(name="pr", bufs=2) as pr,
            tc.tile_pool(name="ph", bufs=2) as ph,
            tc.tile_pool(name="pmono", bufs=2) as pmono,
            tc.tile_pool(name="pb", bufs=3) as pb,
            tc.tile_pool(name="pm", bufs=3) as pm,
            tc.tile_pool(name="ptree", bufs=4) as ptree,
            tc.tile_pool(name="pacc", bufs=2) as pacc,
            tc.tile_pool(name="pout", bufs=2) as pout,
            tc.tile_pool(name="pscr", bufs=2) as pscr,
        ):
            ct = pconst.tile([128, 8], f32)
            nc.sync.dma_start(out=ct[:], in_=consts_ext[:])

            for t in range(NT):
                shin = psh.tile([128, F * 32], bf16, tag="shin")
                shin3 = shin[:].rearrange("p (j f) -> p j f", f=F)
                nc.sync.dma_start(
                    out=shin3[:, 0:16, :],
                    in_=sh_ap[:, 0:16, t * F:(t + 1) * F],
                )
                nc.sync.dma_start(
                    out=shin3[:, 16:32, :],
                    in_=sh_ap[:, 16:32, t * F:(t + 1) * F],
                )
                ctile = pco.tile([128, F * 3], f32, tag="ctile")
                nc.gpsimd.dma_start(
                    out=ctile[:].rearrange("p (c f) -> p c f", f=F),
                    in_=coords_ap[:, :, t * F:(t + 1) * F],
                )

                cv = ctile[:].rearrange("p (c f) -> p c f", c=3)  # plane comps

                # d = coords - rx is precomputed on the host; square all
                # three planes in one ScalarE op
                sq = psq.tile([128, 3 * F], f32, tag="sq", bufs=1)
                nc.scalar.activation(sq[:], ctile[:], AF.Square, bias=0.0,
                                     scale=1.0)

                r2a = pr.tile([128, F], f32, tag="r2a")
                nc.vector.tensor_add(r2a[:], sq[:, 0:F], sq[:, F:2 * F])
                nc.vector.scalar_tensor_tensor(
                    r2a[:], sq[:, 2 * F:3 * F], 1e-12, r2a[:], OP.add, OP.add
                )
                inv = pr.tile([128, F], f32, tag="inv")
                nc.vector.reciprocal_approx_fast(inv[:], r2a[:])
                # sqrt(C1^2 / r2) = C1 * rsqrt(r2)
                rinv = inv
                nc.scalar.activation(rinv[:], inv[:], AF.Sqrt, bias=0.0,
                                     scale=_C12)

                # C1-scaled unit vector: d * rinvC1, rinv broadcast over the
                # three component planes in a single VectorE op
                hats = ph.tile([128, 3 * F], f32, tag="hats")
                nc.vector.tensor_tensor(
                    hats[:].rearrange("p (c f) -> p c f", c=3),
                    ctile[:].rearrange("p (c f) -> p c f", c=3),
                    rinv[:].unsqueeze(1).broadcast_to((128, 3, F)),
                    OP.mult,
                )
                X = hats[:, 0:F]
                Y = hats[:, F:2 * F]
                Z = hats[:, 2 * F:3 * F]

                sqh = ph.tile([128, 3 * F], f32, tag="sqh", bufs=1)
                nc.scalar.activation(sqh[:], hats[:], AF.Square, bias=0.0,
                                     scale=1.0)
                XX = sqh[:, 0:F]
                YY = sqh[:, F:2 * F]
                ZZ = sqh[:, 2 * F:3 * F]

                xy = pmono.tile([128, F], f32, tag="xy")
                nc.vector.tensor_mul(xy[:], X, Y)
                t8 = pmono.tile([128, F], f32, tag="t8")
                nc.vector.tensor_sub(t8[:], XX, YY)
                u9 = pmono.tile([128, F], f32, tag="u9")
                nc.vector.affine_then_add(u9[:], XX, t8[:], 2.0, 0.0)
                u15 = pmono.tile([128, F], f32, tag="u15")
                nc.vector.affine_then_add(u15[:], YY, t8[:], -2.0, 0.0)

                # ---- MAC: pair-batched products ([k2, c, f] = 4F per op) ----
                # Basis planes are packed in pairs matching consecutive k so
                # each product instruction covers 2 k's x 2 channels, and the
                # add tree runs on [4F] chunks (terms halve each level).
                def bpair_tile():
                    return pb.tile([128, 2 * F], bf16, tag="b", name="bp")

                def mk_product2(p_idx, bp):
                    m = pm.tile([128, 4 * F], bf16, tag="m", name="m")
                    in1 = shin[:, 4 * p_idx * F:(4 * p_idx + 4) * F].rearrange(
                        "p (k c f) -> p k c f", k=2, c=2)
                    in0 = bp[:].rearrange("p (k f) -> p k f", k=2) \
                        .unsqueeze(2).broadcast_to((128, 2, 2, F))
                    nc.vector.tensor_tensor(
                        m[:].rearrange("p (k c f) -> p k c f", k=2, c=2),
                        in0, in1, OP.mult)
                    return m

                def amr(out_slice, in0, in1, scale, bias):
                    scr = pscr.tile([128, 1], f32, tag="scr", name="scr")
                    nc.vector.affine_mul_reduce(out_slice, scr[:], in0, in1,
                                                scale, bias)

                def tadd(a, b_, dt):
                    tg = "treeb" if dt == bf16 else "treef"
                    nb = 5 if dt == bf16 else 2
                    o = ptree.tile([128, a.shape[1]], dt, tag=tg, name="tr",
                                   bufs=nb)
                    nc.vector.tensor_tensor(o[:], a[:], b_[:], OP.add)
                    return o

                # hb holds [C0, -Ytilde, +Ztilde, -Xtilde, +Ztilde, -Ytilde]:
                # slots 0-3 are the first two basis pairs; [2F:4F] doubles as
                # the (Z,X) hat pair and [4F:6F] as the (Z,Y) hat pair for the
                # paired deg>=2 plane multiplies.
                hb = pb.tile([128, 6 * F], bf16, tag="hb", name="hb", bufs=2)
                nc.vector.memset(hb[:, 0:F], C0)
                nc.scalar.mul(hb[:, F:2 * F], Y, -1.0)
                nc.scalar.copy(hb[:, 2 * F:3 * F], Z)
                nc.scalar.mul(hb[:, 3 * F:4 * F], X, -1.0)
                nc.scalar.copy(hb[:, 4 * F:5 * F], Z)
                nc.scalar.mul(hb[:, 5 * F:6 * F], Y, -1.0)
                Yn = hb[:, F:2 * F]
                Zb = hb[:, 2 * F:3 * F]
                ZXn = hb[:, 2 * F:4 * F]
                ZYn = hb[:, 4 * F:6 * F]
                m0 = mk_product2(0, hb[:, 0:2 * F])
                m1 = mk_product2(1, hb[:, 2 * F:4 * F])
                A = tadd(m0, m1, bf16)

                # ScalarE pre-scales one factor of each remaining plane to
                # bf16 (constants and signs folded), VectorE finishes with a
                # bf16 2x multiply against a signed hat (pair) plane.
                def spre(out_slice, pre_in, scale, bias=None):
                    if bias is None:
                        nc.scalar.mul(out_slice, pre_in, scale)
                    else:
                        nc.scalar.activation(out_slice, pre_in, AF.Identity,
                                             bias=bias, scale=scale)

                def sb_tile(w):
                    return pb.tile([128, w * F], bf16, tag="sb", name="sb",
                                   bufs=4)

                # pair (k4, k5): [c4*xy, c5*Y*Z]
                bp2 = bpair_tile()
                nc.scalar.mul(bp2[:, 0:F], xy[:], CC4)
                s5 = sb_tile(1)
                spre(s5[:], Y, CC5)
                nc.vector.tensor_mul(bp2[:, F:2 * F], s5[:], Zb)
                m2 = mk_product2(2, bp2)
                # pair (k6, k7): [a6*ZZ+d6, c7*X*Z]
                bp3 = bpair_tile()
                nc.scalar.activation(bp3[:, 0:F], ZZ, AF.Identity,
                                     bias=ct[:, 3:4], scale=A6)
                s7 = sb_tile(1)
                spre(s7[:], X, CC7)
                nc.vector.tensor_mul(bp3[:, F:2 * F], s7[:], Zb)
                m3 = mk_product2(3, bp3)
                B = tadd(m2, m3, bf16)

                # pair (k8, k9): [c8*t8, c9*u9*Y]
                bp4 = bpair_tile()
                nc.scalar.mul(bp4[:, 0:F], t8[:], CC8)
                s9 = sb_tile(1)
                spre(s9[:], u9[:], -CC9)
                nc.vector.tensor_mul(bp4[:, F:2 * F], s9[:], Yn)
                m4 = mk_product2(4, bp4)
                # pair (k10, k11): [c10*xy*Z, (a11*ZZ+d11)*Y] via (Z,Y) pair
                sp5 = sb_tile(2)
                spre(sp5[:, 0:F], xy[:], CC10)
                spre(sp5[:, F:2 * F], ZZ, -A11, bias=ct[:, 4:5])
                bp5 = bpair_tile()
                nc.vector.tensor_mul(bp5[:], sp5[:], ZYn)
                m5 = mk_product2(5, bp5)
                Cc = tadd(m4, m5, bf16)

                # pair (k12, k13): [(a12*ZZ+d12)*Z, (a13*ZZ+d13)*X] via (Z,X)
                sp6 = sb_tile(2)
                spre(sp6[:, 0:F], ZZ, A12, bias=ct[:, 5:6])
                spre(sp6[:, F:2 * F], ZZ, -A13, bias=ct[:, 6:7])
                bp6 = bpair_tile()
                nc.vector.tensor_mul(bp6[:], sp6[:], ZXn)
                m6 = mk_product2(6, bp6)
                # pair (k14, k15): [c14*t8*Z, c15*u15*X] via (Z,X) pair
                sp7 = sb_tile(2)
                spre(sp7[:, 0:F], t8[:], CC14)
                spre(sp7[:, F:2 * F], u15[:], -CC15)
                bp7 = bpair_tile()
                nc.vector.tensor_mul(bp7[:], sp7[:], ZXn)
                m7 = mk_product2(7, bp7)
                D = tadd(m6, m7, bf16)

                E = tadd(A, B, bf16)
                G = tadd(Cc, D, bf16)
                H = tadd(E, G, bf16)
                acc = pacc.tile([128, 2 * F], f32, tag="acc")
                nc.vector.tensor_add(acc[:], H[:, 0:2 * F], H[:, 2 * F:4 * F])

                # interleave back to (f, c) and upcast to fp32
                out_t = pout.tile([128, 2 * F], f32, tag="out")
                nc.scalar.copy(
                    out_t[:].rearrange("p (f c) -> p c f", c=2),
                    acc[:].rearrange("p (c f) -> p c f", c=2),
                )
                nc.gpsimd.dma_start(
                    out=out_ap[:, t * 2 * F:(t + 1) * 2 * F], in_=out_t[:]
                )

    nc.finalize()
    return nc


_NC_CACHE = None
_last_in_maps = None


def _get_nc():
    global _NC_CACHE
    if _NC_CACHE is None:
        _NC_CACHE = _build_nc()
    return _NC_CACHE


def kernel(coordinates, active_deg, max_coeffs, sh_coefficients, rx_pos,
           **unused):
    assert int(active_deg) == ACTIVE_DEG and int(max_coeffs) == K
    coords = np.ascontiguousarray(np.asarray(coordinates, dtype=np.float32))
    sh = np.ascontiguousarray(np.asarray(sh_coefficients, dtype=np.float32))
    rx = np.asarray(rx_pos, dtype=np.float32).reshape(3)
    n = coords.shape[0]
    assert n == N and sh.shape == (N * K, CH)

    consts = np.zeros((128, 8), dtype=np.float32)
    consts[:, 0:3] = -rx[None, :]
    consts[:, 3] = D6
    consts[:, 4] = -D11
    consts[:, 5] = D12
    consts[:, 6] = -D13

    # Host-side relayout: coords -> 3 fp32 planes, sh -> 32 bf16 (k,c)-planes,
    # so device DMAs land directly in compute layout.
    sh32 = sh.reshape(n, K * CH)
    coordsT = coords.T - rx[:, None]  # [3, N], receiver offset folded in
    in_maps = []
    for c in range(NCORES):
        lo, hi = c * PC, (c + 1) * PC
        real = min(hi, n) - lo
        coords_c = np.zeros((3, PC), dtype=np.float32)
        coords_c[:, :real] = coordsT[:, lo:lo + real]
        sh_c = np.zeros((32, PC), dtype=ml_dtypes.bfloat16)
        sh_c[:, :real] = sh32[lo:lo + real].T
        in_maps.append({"coords": coords_c, "sh": sh_c, "consts": consts})

    global _last_in_maps
    _last_in_maps = in_maps
    res = run_bass_kernel_spmd(_get_nc(), in_maps, list(range(NCORES)))
    out = np.concatenate([np.asarray(res.results[c]["out"])
                          for c in range(NCORES)], axis=0)
    return out[:n]


# revision 34
# speedup vs baseline: 1.0051x; 1.0027x over previous
"""Trainium2 Bass kernel for degree-3 real spherical-harmonics evaluation.

Computes, for N=2M points with 16 SH coefficients x 2 channels each:
    d    = normalize(coordinates - rx_pos)
    out  = sum_k basis_k(d) * sh[n, k, c]

Strategy (8 NeuronCores, data-parallel over points):
  - Pad N to 2,007,040 = 8 cores * 128 partitions * 1960 points; each core
    gets a contiguous slab (point n -> partition n//1960, column n%1960).
  - Host-side relayout inside kernel(): coords become 3 fp32 planes with
    rx_pos pre-subtracted; sh becomes 32 (k,c)-planes cast to bf16. Every
    DMA then lands in the exact SBUF layout compute wants (contiguous
    per-plane runs >= 512B per descriptor) and sh DRAM traffic is halved.
  - Normalization: r^2 via one ScalarE Square + two VectorE adds,
    reciprocal_approx_fast (DVE) + ScalarE Sqrt with the C1 SH constant
    folded into its scale, so the unit vector arrives pre-scaled by C1.
  - Basis: unit-sphere identities collapse deg-2/3 polynomials to affine
    forms of z^2; ScalarE pre-scales one factor of each plane to bf16 with
    all SH constants/signs folded; VectorE finishes with bf16 2x-mode
    multiplies against signed hat planes, packed in pairs so one [2F] op
    builds two basis planes.
  - MAC: 8 rank-4 broadcast tensor_tensor products (2 k's x 2 channels per
    instruction, bf16 2x mode) + a binary add tree on [4F] chunks (bf16
    except the final fp32 fold), then ScalarE re-interleaves/upcasts to the
    fp32 [N,2] output.
"""

import ml_dtypes
import numpy as np

import concourse.bass as bass
import concourse.tile as tile
from concourse import bacc, mybir
from concourse.bass_utils import run_bass_kernel_spmd

f32 = mybir.dt.float32
bf16 = mybir.dt.bfloat16
AF = mybir.ActivationFunctionType
OP = mybir.AluOpType

# ----- problem constants (hardcoded per spec) -----
N = 2_000_000
K = 16
CH = 2
ACTIVE_DEG = 3

C0 = 0.28209479177387814
C1 = 0.4886025119029199
C2 = (1.0925484305920792, -1.0925484305920792, 0.31539156525252005,
      -1.0925484305920792, 0.5462742152960396)
C3 = (-0.5900435899266435, 2.890611442640554, -0.4570457994644658,
      0.3731763325901154, -0.4570457994644658, 1.445305721320277,
      -0.5900435899266435)

# Basis constants with the C1 hat-scaling folded in (hats carry a factor C1).
_C12 = C1 * C1
_C13 = C1 * C1 * C1
CC4 = C2[0] / _C12
CC5 = C2[1] / _C12
A6, D6 = 3.0 * C2[2] / _C12, -C2[2]
CC7 = C2[3] / _C12
CC8 = C2[4] / _C12
CC9 = C3[0] / _C13
CC10 = C3[1] / _C13
A11, D11 = 5.0 * C3[2] / _C13, -C3[2] / C1
A12, D12 = 5.0 * C3[3] / _C13, -3.0 * C3[3] / C1
A13, D13 = 5.0 * C3[4] / _C13, -C3[4] / C1
CC14 = C3[5] / _C13
CC15 = C3[6] / _C13

# ----- sharding geometry -----
NCORES = 8
PPART = 1960                 # points per partition per core
PC = 128 * PPART             # points per core = 250,880
NPAD = NCORES * PC           # 2,007,040
TF = 490                     # points per partition per tile
NT = PPART // TF             # 5 tiles


def _build_nc():
    # Inputs arrive host-preprocessed: coords as 3 planes [3, PC] fp32 and
    # sh as 32 (k,c)-planes [32, PC] bf16 — every DMA lands in the exact
    # SBUF layout compute wants, no on-chip shuffling.
    nc = bacc.Bacc("TRN2")
    coords_ext = nc.declare_dram_parameter("coords", [3, PC], f32, isOutput=False)
    sh_ext = nc.declare_dram_parameter("sh", [32, PC], bf16, isOutput=False)
    consts_ext = nc.declare_dram_parameter("consts", [128, 8], f32, isOutput=False)
    out_ext = nc.declare_dram_parameter("out", [PC, CH], f32, isOutput=True)

    coords_ap = coords_ext[:].rearrange("c (p f) -> p c f", p=128)   # [128,3,1960]
    sh_ap = sh_ext[:].rearrange("j (p f) -> p j f", p=128)           # [128,32,1960]
    out_ap = out_ext[:].rearrange("(p f) c -> p (f c)", p=128)       # [128, 3920]

    F = TF
    with tile.TileContext(nc) as tc:
        with (
            tc.tile_pool(name="pconst", bufs=1) as pconst,
            tc.tile_pool(name="psh", bufs=2) as psh,
            tc.tile_pool(name="pco", bufs=2) as pco,
            tc.tile_pool(name="psq", bufs=2) as psq,
            tc.tile_pool(name="pr", bufs=2) as pr,
            tc.tile_pool(name="ph", bufs=2) as ph,
            tc.tile_pool(name="pmono", bufs=2) as pmono,
            tc.tile_pool(name="pb", bufs=3) as pb,
            tc.tile_pool(name="pm", bufs=3) as pm,
            tc.tile_pool(name="ptree", bufs=4) as ptree,
            tc.tile_pool(name="pacc", bufs=2) as pacc,
            tc.tile_pool(name="pout", bufs=2) as pout,
            tc.tile_pool(name="pscr", bufs=2) as pscr,
        ):
            ct = pconst.tile([128, 8], f32)
            nc.sync.dma_start(out=ct[:], in_=consts_ext[:])

            for t in range(NT):
                shin = psh.tile([128, F * 32], bf16, tag="shin")
                shin3 = shin[:].rearrange("p (j f) -> p j f", f=F)
                nc.sync.dma_start(
                    out=shin3[:, 0:16, :],
                    in_=sh_ap[:, 0:16, t * F:(t + 1) * F],
                )
                nc.sync.dma_start(
                    out=shin3[:, 16:32, :],
                    in_=sh_ap[:, 16:32, t * F:(t + 1) * F],
                )
                ctile = pco.tile([128, F * 3], f32, tag="ctile")
                nc.gpsimd.dma_start(
                    out=ctile[:].rearrange("p (c f) -> p c f", f=F),
                    in_=coords_ap[:, :, t * F:(t + 1) * F],
                )

                cv = ctile[:].rearrange("p (c f) -> p c f", c=3)  # plane comps

                # d = coords - rx is precomputed on the host; square all
                # three planes in one ScalarE op
                sq = psq.tile([128, 3 * F], f32, tag="sq", bufs=1)
                nc.scalar.activation(sq[:], ctile[:], AF.Square, bias=0.0,
                                     scale=1.0)

                r2a = pr.tile([128, F], f32, tag="r2a")
                nc.vector.tensor_add(r2a[:], sq[:, 0:F], sq[:, F:2 * F])
                nc.vector.scalar_tensor_tensor(
                    r2a[:], sq[:, 2 * F:3 * F], 1e-12, r2a[:], OP.add, OP.add
                )
                inv = pr.tile([128, F], f32, tag="inv")
                nc.vector.reciprocal_approx_fast(inv[:], r2a[:])
                # sqrt(C1^2 / r2) = C1 * rsqrt(r2)
                rinv = inv
                nc.scalar.activation(rinv[:], inv[:], AF.Sqrt, bias=0.0,
                                     scale=_C12)

                # C1-scaled unit vector: d * rinvC1, rinv broadcast over the
                # three component planes in a single VectorE op
                hats = ph.tile([128, 3 * F], f32, tag="hats")
                nc.vector.tensor_tensor(
                    hats[:].rearrange("p (c f) -> p c f", c=3),
                    ctile[:].rearrange("p (c f) -> p c f", c=3),
                    rinv[:].unsqueeze(1).broadcast_to((128, 3, F)),
                    OP.mult,
                )
                X = hats[:, 0:F]
                Y = hats[:, F:2 * F]
                Z = hats[:, 2 * F:3 * F]

                sqh = ph.tile([128, 3 * F], f32, tag="sqh", bufs=1)
                nc.scalar.activation(sqh[:], hats[:], AF.Square, bias=0.0,
                                     scale=1.0)
                XX = sqh[:, 0:F]
                YY = sqh[:, F:2 * F]
                ZZ = sqh[:, 2 * F:3 * F]

                xy = pmono.tile([128, F], f32, tag="xy")
                nc.vector.tensor_mul(xy[:], X, Y)
                t8 = pmono.tile([128, F], f32, tag="t8")
                nc.vector.tensor_sub(t8[:], XX, YY)
                u9 = pmono.tile([128, F], f32, tag="u9")
                nc.vector.affine_then_add(u9[:], XX, t8[:], 2.0, 0.0)
                u15 = pmono.tile([128, F], f32, tag="u15")
                nc.vector.affine_then_add(u15[:], YY, t8[:], -2.0, 0.0)

                # ---- MAC: pair-batched products ([k2, c, f] = 4F per op) ----
                # Basis planes are packed in pairs matching consecutive k so
                # each product instruction covers 2 k's x 2 channels, and the
                # add tree runs on [4F] chunks (terms halve each level).
                def bpair_tile():
                    return pb.tile([128, 2 * F], bf16, tag="b", name="bp")

                def mk_product2(p_idx, bp):
                    m = pm.tile([128, 4 * F], bf16, tag="m", name="m")
                    in1 = shin[:, 4 * p_idx * F:(4 * p_idx + 4) * F].rearrange(
                        "p (k c f) -> p k c f", k=2, c=2)
                    in0 = bp[:].rearrange("p (k f) -> p k f", k=2) \
                        .unsqueeze(2).broadcast_to((128, 2, 2, F))
                    nc.vector.tensor_tensor(
                        m[:].rearrange("p (k c f) -> p k c f", k=2, c=2),
                        in0, in1, OP.mult)
                    return m

                def amr(out_slice, in0, in1, scale, bias):
                    scr = pscr.tile([128, 1], f32, tag="scr", name="scr")
                    nc.vector.affine_mul_reduce(out_slice, scr[:], in0, in1,
                                                scale, bias)

                def tadd(a, b_, dt):
                    tg = "treeb" if dt == bf16 else "treef"
                    nb = 5 if dt == bf16 else 2
                    o = ptree.tile([128, a.shape[1]], dt, tag=tg, name="tr",
                                   bufs=nb)
                    nc.vector.tensor_tensor(o[:], a[:], b_[:], OP.add)
                    return o

                # hb holds [C0, -Ytilde, +Ztilde, -Xtilde, +Ztilde, -Ytilde]:
                # slots 0-3 are the first two basis pairs; [2F:4F] doubles as
                # the (Z,X) hat pair and [4F:6F] as the (Z,Y) hat pair for the
                # paired deg>=2 plane multiplies.
                hb = pb.tile([128, 6 * F], bf16, tag="hb", name="hb", bufs=2)
                nc.vector.memset(hb[:, 0:F], C0)
                nc.scalar.mul(hb[:, F:2 * F], Y, -1.0)
                nc.scalar.copy(hb[:, 2 * F:3 * F], Z)
                nc.scalar.mul(hb[:, 3 * F:4 * F], X, -1.0)
                nc.scalar.copy(hb[:, 4 * F:5 * F], Z)
                nc.scalar.mul(hb[:, 5 * F:6 * F], Y, -1.0)
                Yn = hb[:, F:2 * F]
                Zb = hb[:, 2 * F:3 * F]
                ZXn = hb[:, 2 * F:4 * F]
                ZYn = hb[:, 4 * F:6 * F]
                m0 = mk_product2(0, hb[:, 0:2 * F])
                m1 = mk_product2(1, hb[:, 2 * F:4 * F])
                A = tadd(m0, m1, bf16)

                # ScalarE pre-scales one factor of each remaining plane to
                # bf16 (constants and signs folded), VectorE finishes with a
                # bf16 2x multiply against a signed hat (pair) plane.
                def spre(out_slice, pre_in, scale, bias=None):
                    if bias is None:
                        nc.scalar.mul(out_slice, pre_in, scale)
                    else:
                        nc.scalar.activation(out_slice, pre_in, AF.Identity,
                                             bias=bias, scale=scale)

                def sb_tile(w):
                    return pb.tile([128, w * F], bf16, tag="sb", name="sb",
                                   bufs=4)

                # pair (k4, k5): [c4*xy, c5*Y*Z]
                bp2 = bpair_tile()
                nc.scalar.mul(bp2[:, 0:F], xy[:], CC4)
                s5 = sb_tile(1)
                spre(s5[:], Y, CC5)
                nc.vector.tensor_mul(bp2[:, F:2 * F], s5[:], Zb)
                m2 = mk_product2(2, bp2)
                # pair (k6, k7): [a6*ZZ+d6, c7*X*Z]
                bp3 = bpair_tile()
                nc.scalar.activation(bp3[:, 0:F], ZZ, AF.Identity,
                                     bias=ct[:, 3:4], scale=A6)
                s7 = sb_tile(1)
                spre(s7[:], X, CC7)
                nc.vector.tensor_mul(bp3[:, F:2 * F], s7[:], Zb)
                m3 = mk_product2(3, bp3)
                B = tadd(m2, m3, bf16)

                # pair (k8, k9): [c8*t8, c9*u9*Y]
                bp4 = bpair_tile()
                nc.scalar.mul(bp4[:, 0:F], t8[:], CC8)
                s9 = sb_tile(1)
                spre(s9[:], u9[:], -CC9)
                nc.vector.tensor_mul(bp4[:, F:2 * F], s9[:], Yn)
                m4 = mk_product2(4, bp4)
                # pair (k10, k11): [c10*xy*Z, (a11*ZZ+d11)*Y] via (Z,Y) pair
                sp5 = sb_tile(2)
                spre(sp5[:, 0:F], xy[:], CC10)
                spre(sp5[:, F:2 * F], ZZ, -A11, bias=ct[:, 4:5])
                bp5 = bpair_tile()
                nc.vector.tensor_mul(bp5[:], sp5[:], ZYn)
                m5 = mk_product2(5, bp5)
                Cc = tadd(m4, m5, bf16)

                # pair (k12, k13): [(a12*ZZ+d12)*Z, (a13*ZZ+d13)*X] via (Z,X)
                sp6 = sb_tile(2)
                spre(sp6[:, 0:F], ZZ, A12, bias=ct[:, 5:6])
                spre(sp6[:, F:2 * F], ZZ, -A13, bias=ct[:, 6:7])
                bp6 = bpair_tile()
                nc.vector.tensor_mul(bp6[:], sp6[:], ZXn)
                m6 = mk_product2(6, bp6)
                # pair (k14, k15): [c14*t8*Z, c15*u15*X] via (Z,X) pair
                sp7 = sb_tile(2)
                spre(sp7[:, 0:F], t8[:], CC14)
                spre(sp7[:, F:2 * F], u15[:], -CC15)
                bp7 = bpair_tile()
                nc.vector.tensor_mul(bp7[:], sp7[:], ZXn)
                m7 = mk_product2(7, bp7)
                D = tadd(m6, m7, bf16)

                E = tadd(A, B, bf16)
                G = tadd(Cc, D, bf16)
                H = tadd(E, G, bf16)

                # final fold writes the interleaved fp32 output directly
                out_t = pout.tile([128, 2 * F], f32, tag="out")
                nc.vector.tensor_tensor(
                    out_t[:].rearrange("p (f c) -> p c f", c=2),
                    H[:, 0:2 * F].rearrange("p (c f) -> p c f", c=2),
                    H[:, 2 * F:4 * F].rearrange("p (c f) -> p c f", c=2),
                    OP.add,
                )
                nc.gpsimd.dma_start(
                    out=out_ap[:, t * 2 * F:(t + 1) * 2 * F], in_=out_t[:]
                )

    nc.finalize()
    return nc


_NC_CACHE = None
_last_in_maps = None


def _get_nc():
    global _NC_CACHE
    if _NC_CACHE is None:
        _NC_CACHE = _build_nc()
    return _NC_CACHE


def kernel(coordinates, active_deg, max_coeffs, sh_coefficients, rx_pos,
           **unused):
    assert int(active_deg) == ACTIVE_DEG and int(max_coeffs) == K
    coords = np.ascontiguousarray(np.asarray(coordinates, dtype=np.float32))
    sh = np.ascontiguousarray(np.asarray(sh_coefficients, dtype=np.float32))
    rx = np.asarray(rx_pos, dtype=np.float32).reshape(3)
    n = coords.shape[0]
    assert n == N and sh.shape == (N * K, CH)

    consts = np.zeros((128, 8), dtype=np.float32)
    consts[:, 0:3] = -rx[None, :]
    consts[:, 3] = D6
    consts[:, 4] = -D11
    consts[:, 5] = D12
    consts[:, 6] = -D13

    # Host-side relayout: coords -> 3 fp32 planes, sh -> 32 bf16 (k,c)-planes,
    # so device DMAs land directly in compute layout.
    sh32 = sh.reshape(n, K * CH)
    coordsT = coords.T - rx[:, None]  # [3, N], receiver offset folded in
    in_maps = []
    for c in range(NCORES):
        lo, hi = c * PC, (c + 1) * PC
        real = min(hi, n) - lo
        coords_c = np.zeros((3, PC), dtype=np.float32)
        coords_c[:, :real] = coordsT[:, lo:lo + real]
        sh_c = np.zeros((32, PC), dtype=ml_dtypes.bfloat16)
        sh_c[:, :real] = sh32[lo:lo + real].T
        in_maps.append({"coords": coords_c, "sh": sh_c, "consts": consts})

    global _last_in_maps
    _last_in_maps = in_maps
    res = run_bass_kernel_spmd(_get_nc(), in_maps, list(range(NCORES)))
    out = np.concatenate([np.asarray(res.results[c]["out"])
                          for c in range(NCORES)], axis=0)
    return out[:n]


# revision 36
# speedup vs baseline: 1.0162x; 1.0110x over previous
"""Trainium2 Bass kernel for degree-3 real spherical-harmonics evaluation.

Computes, for N=2M points with 16 SH coefficients x 2 channels each:
    d    = normalize(coordinates - rx_pos)
    out  = sum_k basis_k(d) * sh[n, k, c]

Strategy (8 NeuronCores, data-parallel over points):
  - Pad N to 2,007,040 = 8 cores * 128 partitions * 1960 points; each core
    gets a contiguous slab (point n -> partition n//1960, column n%1960).
  - Host-side relayout inside kernel(): coords become 3 fp32 planes with
    rx_pos pre-subtracted; sh becomes 32 (k,c)-planes cast to bf16. Every
    DMA then lands in the exact SBUF layout compute wants (contiguous
    per-plane runs >= 512B per descriptor) and sh DRAM traffic is halved.
  - Normalization: r^2 via one ScalarE Square + two VectorE adds,
    reciprocal_approx_fast (DVE) + ScalarE Sqrt with the C1 SH constant
    folded into its scale, so the unit vector arrives pre-scaled by C1.
  - Basis: unit-sphere identities collapse deg-2/3 polynomials to affine
    forms of z^2; ScalarE pre-scales one factor of each plane to bf16 with
    all SH constants/signs folded; VectorE finishes with bf16 2x-mode
    multiplies against signed hat planes, packed in pairs so one [2F] op
    builds two basis planes.
  - MAC: 8 rank-4 broadcast tensor_tensor products (2 k's x 2 channels per
    instruction, bf16 2x mode) + a binary add tree on [4F] chunks (bf16
    except the final fp32 fold), then ScalarE re-interleaves/upcasts to the
    fp32 [N,2] output.
"""

import ml_dtypes
import numpy as np

import concourse.bass as bass
import concourse.tile as tile
from concourse import bacc, mybir
from concourse.bass_utils import run_bass_kernel_spmd

f32 = mybir.dt.float32
bf16 = mybir.dt.bfloat16
AF = mybir.ActivationFunctionType
OP = mybir.AluOpType

# ----- problem constants (hardcoded per spec) -----
N = 2_000_000
K = 16
CH = 2
ACTIVE_DEG = 3

C0 = 0.28209479177387814
C1 = 0.4886025119029199
C2 = (1.0925484305920792, -1.0925484305920792, 0.31539156525252005,
      -1.0925484305920792, 0.5462742152960396)
C3 = (-0.5900435899266435, 2.890611442640554, -0.4570457994644658,
      0.3731763325901154, -0.4570457994644658, 1.445305721320277,
      -0.5900435899266435)

# Basis constants with the C1 hat-scaling folded in (hats carry a factor C1).
_C12 = C1 * C1
_C13 = C1 * C1 * C1
CC4 = C2[0] / _C12
CC5 = C2[1] / _C12
A6, D6 = 3.0 * C2[2] / _C12, -C2[2]
CC7 = C2[3] / _C12
CC8 = C2[4] / _C12
CC9 = C3[0] / _C13
CC10 = C3[1] / _C13
A11, D11 = 5.0 * C3[2] / _C13, -C3[2] / C1
A12, D12 = 5.0 * C3[3] / _C13, -3.0 * C3[3] / C1
A13, D13 = 5.0 * C3[4] / _C13, -C3[4] / C1
CC14 = C3[5] / _C13
CC15 = C3[6] / _C13

# ----- sharding geometry -----
NCORES = 8
PPART = 1960                 # points per partition per core
PC = 128 * PPART             # points per core = 250,880
NPAD = NCORES * PC           # 2,007,040
TF = 490                     # points per partition per tile
NT = PPART // TF             # 5 tiles


def _build_nc():
    # Inputs arrive host-preprocessed: coords as 3 planes [3, PC] fp32 and
    # sh as 32 (k,c)-planes [32, PC] bf16 — every DMA lands in the exact
    # SBUF layout compute wants, no on-chip shuffling.
    nc = bacc.Bacc("TRN2")
    coords_ext = nc.declare_dram_parameter("coords", [3, PC], f32, isOutput=False)
    sh_ext = nc.declare_dram_parameter("sh", [32, PC], bf16, isOutput=False)
    consts_ext = nc.declare_dram_parameter("consts", [128, 8], f32, isOutput=False)
    out_ext = nc.declare_dram_parameter("out", [PC, CH], f32, isOutput=True)

    coords_ap = coords_ext[:].rearrange("c (p f) -> p c f", p=128)   # [128,3,1960]
    sh_ap = sh_ext[:].rearrange("j (p f) -> p j f", p=128)           # [128,32,1960]
    out_ap = out_ext[:].rearrange("(p f) c -> p (f c)", p=128)       # [128, 3920]

    F = TF
    with tile.TileContext(nc) as tc:
        with (
            tc.tile_pool(name="pconst", bufs=1) as pconst,
            tc.tile_pool(name="psh", bufs=2) as psh,
            tc.tile_pool(name="pco", bufs=2) as pco,
            tc.tile_pool(name="psq", bufs=2) as psq,
            tc.tile_pool(name="pr", bufs=2) as pr,
            tc.tile_pool(name="ph", bufs=2) as ph,
            tc.tile_pool(name="pmono", bufs=2) as pmono,
            tc.tile_pool(name="pb", bufs=3) as pb,
            tc.tile_pool(name="pm", bufs=3) as pm,
            tc.tile_pool(name="ptree", bufs=4) as ptree,
            tc.tile_pool(name="pacc", bufs=2) as pacc,
            tc.tile_pool(name="pout", bufs=2) as pout,
            tc.tile_pool(name="pscr", bufs=2) as pscr,
        ):
            ct = pconst.tile([128, 8], f32)
            nc.sync.dma_start(out=ct[:], in_=consts_ext[:])

            for t in range(NT):
                shin = psh.tile([128, F * 32], bf16, tag="shin")
                shin3 = shin[:].rearrange("p (j f) -> p j f", f=F)
                nc.sync.dma_start(
                    out=shin3[:, 0:16, :],
                    in_=sh_ap[:, 0:16, t * F:(t + 1) * F],
                )
                nc.sync.dma_start(
                    out=shin3[:, 16:32, :],
                    in_=sh_ap[:, 16:32, t * F:(t + 1) * F],
                )
                ctile = pco.tile([128, F * 3], f32, tag="ctile")
                nc.gpsimd.dma_start(
                    out=ctile[:].rearrange("p (c f) -> p c f", f=F),
                    in_=coords_ap[:, :, t * F:(t + 1) * F],
                )

                cv = ctile[:].rearrange("p (c f) -> p c f", c=3)  # plane comps

                # d = coords - rx is precomputed on the host; square all
                # three planes in one ScalarE op
                sq = psq.tile([128, 3 * F], f32, tag="sq", bufs=1)
                nc.scalar.activation(sq[:], ctile[:], AF.Square, bias=0.0,
                                     scale=1.0)

                r2a = pr.tile([128, F], f32, tag="r2a")
                nc.vector.tensor_add(r2a[:], sq[:, 0:F], sq[:, F:2 * F])
                nc.vector.scalar_tensor_tensor(
                    r2a[:], sq[:, 2 * F:3 * F], 1e-12, r2a[:], OP.add, OP.add
                )
                inv = pr.tile([128, F], f32, tag="inv")
                nc.vector.reciprocal_approx_fast(inv[:], r2a[:])
                # sqrt(C1^2 / r2) = C1 * rsqrt(r2)
                rinv = inv
                nc.scalar.activation(rinv[:], inv[:], AF.Sqrt, bias=0.0,
                                     scale=_C12)

                # C1-scaled unit vector: d * rinvC1, rinv broadcast over the
                # three component planes in a single VectorE op
                hats = ph.tile([128, 3 * F], f32, tag="hats")
                nc.vector.tensor_tensor(
                    hats[:].rearrange("p (c f) -> p c f", c=3),
                    ctile[:].rearrange("p (c f) -> p c f", c=3),
                    rinv[:].unsqueeze(1).broadcast_to((128, 3, F)),
                    OP.mult,
                )
                X = hats[:, 0:F]
                Y = hats[:, F:2 * F]
                Z = hats[:, 2 * F:3 * F]

                sqh = ph.tile([128, 3 * F], f32, tag="sqh", bufs=1)
                nc.scalar.activation(sqh[:], hats[:], AF.Square, bias=0.0,
                                     scale=1.0)
                XX = sqh[:, 0:F]
                YY = sqh[:, F:2 * F]
                ZZ = sqh[:, 2 * F:3 * F]

                xy = pmono.tile([128, F], f32, tag="xy")
                nc.vector.tensor_mul(xy[:], X, Y)
                t8 = pmono.tile([128, F], f32, tag="t8")
                nc.vector.tensor_sub(t8[:], XX, YY)
                u9 = pmono.tile([128, F], f32, tag="u9")
                nc.vector.affine_then_add(u9[:], XX, t8[:], 2.0, 0.0)
                u15 = pmono.tile([128, F], f32, tag="u15")
                nc.vector.affine_then_add(u15[:], YY, t8[:], -2.0, 0.0)

                # ---- MAC: quad-batched products ([k4, c, f] = 8F per op) ----
                # Basis planes are packed 4 k's to a tile so each product
                # instruction covers 4 k's x 2 channels; the add tree then
                # needs only 3 [8F] adds + 2 folds.
                def bquad_tile():
                    return pb.tile([128, 4 * F], bf16, tag="b", name="bq")

                def mk_product4(q_idx, bq):
                    m = pm.tile([128, 8 * F], bf16, tag="m", name="m")
                    in1 = shin[:, 8 * q_idx * F:(8 * q_idx + 8) * F].rearrange(
                        "p (k c f) -> p k c f", k=4, c=2)
                    in0 = bq[:].rearrange("p (k f) -> p k f", k=4) \
                        .unsqueeze(2).broadcast_to((128, 4, 2, F))
                    nc.vector.tensor_tensor(
                        m[:].rearrange("p (k c f) -> p k c f", k=4, c=2),
                        in0, in1, OP.mult)
                    return m

                def tadd(a, b_, dt, nb):
                    o = ptree.tile([128, a.shape[1]], dt, tag="tree", name="tr",
                                   bufs=nb)
                    nc.vector.tensor_tensor(o[:], a[:], b_[:], OP.add)
                    return o

                # hb holds [C0, -Ytilde, +Ztilde, -Xtilde, +Ztilde, -Ytilde]:
                # [0:4F] is the k0-k3 basis quad; [2F:4F] doubles as the (Z,X)
                # hat pair and [4F:6F] is the (Z,Y) hat pair for the paired
                # deg>=2 plane multiplies.
                hb = pb.tile([128, 6 * F], bf16, tag="hb", name="hb", bufs=2)
                nc.vector.memset(hb[:, 0:F], C0)
                nc.scalar.mul(hb[:, F:2 * F], Y, -1.0)
                nc.scalar.copy(hb[:, 2 * F:3 * F], Z)
                nc.scalar.mul(hb[:, 3 * F:4 * F], X, -1.0)
                nc.scalar.copy(hb[:, 4 * F:5 * F], Z)
                nc.scalar.mul(hb[:, 5 * F:6 * F], Y, -1.0)
                Yn = hb[:, F:2 * F]
                Zb = hb[:, 2 * F:3 * F]
                ZXn = hb[:, 2 * F:4 * F]
                ZYn = hb[:, 4 * F:6 * F]
                m0 = mk_product4(0, hb[:, 0:4 * F])

                # ScalarE pre-scales one factor of each remaining plane to
                # bf16 (constants and signs folded), VectorE finishes with a
                # bf16 2x multiply against a signed hat (pair) plane.
                def spre(out_slice, pre_in, scale, bias=None):
                    if bias is None:
                        nc.scalar.mul(out_slice, pre_in, scale)
                    else:
                        nc.scalar.activation(out_slice, pre_in, AF.Identity,
                                             bias=bias, scale=scale)

                def sb_tile(w):
                    return pb.tile([128, w * F], bf16, tag="sb", name="sb",
                                   bufs=4)

                # quad (k4..k7): [c4*xy, c5*Y*Z, a6*ZZ+d6, c7*X*Z]
                bq1 = bquad_tile()
                nc.scalar.mul(bq1[:, 0:F], xy[:], CC4)
                s5 = sb_tile(1)
                spre(s5[:], Y, CC5)
                nc.vector.tensor_mul(bq1[:, F:2 * F], s5[:], Zb)
                nc.scalar.activation(bq1[:, 2 * F:3 * F], ZZ, AF.Identity,
                                     bias=ct[:, 3:4], scale=A6)
                s7 = sb_tile(1)
                spre(s7[:], X, CC7)
                nc.vector.tensor_mul(bq1[:, 3 * F:4 * F], s7[:], Zb)
                m1 = mk_product4(1, bq1)
                A = tadd(m0, m1, bf16, 3)

                # quad (k8..k11): [c8*t8, c9*u9*Y, c10*xy*Z, (a11*ZZ+d11)*Y]
                bq2 = bquad_tile()
                nc.scalar.mul(bq2[:, 0:F], t8[:], CC8)
                s9 = sb_tile(1)
                spre(s9[:], u9[:], -CC9)
                nc.vector.tensor_mul(bq2[:, F:2 * F], s9[:], Yn)
                sp5 = sb_tile(2)
                spre(sp5[:, 0:F], xy[:], CC10)
                spre(sp5[:, F:2 * F], ZZ, -A11, bias=ct[:, 4:5])
                nc.vector.tensor_mul(bq2[:, 2 * F:4 * F], sp5[:], ZYn)
                m2 = mk_product4(2, bq2)

                # quad (k12..k15): [(a12*ZZ+d12)*Z, (a13*ZZ+d13)*X,
                #                   c14*t8*Z, c15*u15*X]
                bq3 = bquad_tile()
                sp6 = sb_tile(2)
                spre(sp6[:, 0:F], ZZ, A12, bias=ct[:, 5:6])
                spre(sp6[:, F:2 * F], ZZ, -A13, bias=ct[:, 6:7])
                nc.vector.tensor_mul(bq3[:, 0:2 * F], sp6[:], ZXn)
                sp7 = sb_tile(2)
                spre(sp7[:, 0:F], t8[:], CC14)
                spre(sp7[:, F:2 * F], u15[:], -CC15)
                nc.vector.tensor_mul(bq3[:, 2 * F:4 * F], sp7[:], ZXn)
                m3 = mk_product4(3, bq3)
                B = tadd(m2, m3, bf16, 3)

                H8 = tadd(A, B, bf16, 3)
                H4 = ptree.tile([128, 4 * F], bf16, tag="tree", name="h4",
                                bufs=3)
                nc.vector.tensor_add(H4[:], H8[:, 0:4 * F], H8[:, 4 * F:8 * F])

                # final fold writes the interleaved fp32 output directly
                out_t = pout.tile([128, 2 * F], f32, tag="out")
                nc.vector.tensor_tensor(
                    out_t[:].rearrange("p (f c) -> p c f", c=2),
                    H4[:, 0:2 * F].rearrange("p (c f) -> p c f", c=2),
                    H4[:, 2 * F:4 * F].rearrange("p (c f) -> p c f", c=2),
                    OP.add,
                )
                nc.gpsimd.dma_start(
                    out=out_ap[:, t * 2 * F:(t + 1) * 2 * F], in_=out_t[:]
                )

    nc.finalize()
    return nc


_NC_CACHE = None
_last_in_maps = None


def _get_nc():
    global _NC_CACHE
    if _NC_CACHE is None:
        _NC_CACHE = _build_nc()
    return _NC_CACHE


def kernel(coordinates, active_deg, max_coeffs, sh_coefficients, rx_pos,
           **unused):
    assert int(active_deg) == ACTIVE_DEG and int(max_coeffs) == K
    coords = np.ascontiguousarray(np.asarray(coordinates, dtype=np.float32))
    sh = np.ascontiguousarray(np.asarray(sh_coefficients, dtype=np.float32))
    rx = np.asarray(rx_pos, dtype=np.float32).reshape(3)
    n = coords.shape[0]
    assert n == N and sh.shape == (N * K, CH)

    consts = np.zeros((128, 8), dtype=np.float32)
    consts[:, 0:3] = -rx[None, :]
    consts[:, 3] = D6
    consts[:, 4] = -D11
    consts[:, 5] = D12
    consts[:, 6] = -D13

    # Host-side relayout: coords -> 3 fp32 planes, sh -> 32 bf16 (k,c)-planes,
    # so device DMAs land directly in compute layout.
    sh32 = sh.reshape(n, K * CH)
    coordsT = coords.T - rx[:, None]  # [3, N], receiver offset folded in
    in_maps = []
    for c in range(NCORES):
        lo, hi = c * PC, (c + 1) * PC
        real = min(hi, n) - lo
        coords_c = np.zeros((3, PC), dtype=np.float32)
        coords_c[:, :real] = coordsT[:, lo:lo + real]
        sh_c = np.zeros((32, PC), dtype=ml_dtypes.bfloat16)
        sh_c[:, :real] = sh32[lo:lo + real].T
        in_maps.append({"coords": coords_c, "sh": sh_c, "consts": consts})

    global _last_in_maps
    _last_in_maps = in_maps
    res = run_bass_kernel_spmd(_get_nc(), in_maps, list(range(NCORES)))
    out = np.concatenate([np.asarray(res.results[c]["out"])
                          for c in range(NCORES)], axis=0)
    return out[:n]


# revision 37
# speedup vs baseline: 1.0249x; 1.0086x over previous
"""Trainium2 Bass kernel for degree-3 real spherical-harmonics evaluation.

Computes, for N=2M points with 16 SH coefficients x 2 channels each:
    d    = normalize(coordinates - rx_pos)
    out  = sum_k basis_k(d) * sh[n, k, c]

Strategy (8 NeuronCores, data-parallel over points):
  - Pad N to 2,007,040 = 8 cores * 128 partitions * 1960 points; each core
    gets a contiguous slab (point n -> partition n//1960, column n%1960).
  - Host-side relayout inside kernel(): coords become 3 fp32 planes with
    rx_pos pre-subtracted; sh becomes 32 (k,c)-planes cast to bf16. Every
    DMA then lands in the exact SBUF layout compute wants (contiguous
    per-plane runs >= 512B per descriptor) and sh DRAM traffic is halved.
  - Normalization: r^2 via one ScalarE Square + two VectorE adds,
    reciprocal_approx_fast (DVE) + ScalarE Sqrt with the C1 SH constant
    folded into its scale, so the unit vector arrives pre-scaled by C1.
  - Basis: unit-sphere identities collapse deg-2/3 polynomials to affine
    forms of z^2; ScalarE pre-scales one factor of each plane to bf16 with
    all SH constants/signs folded; VectorE finishes with bf16 2x-mode
    multiplies against signed hat planes, packed in pairs so one [2F] op
    builds two basis planes.
  - MAC: 8 rank-4 broadcast tensor_tensor products (2 k's x 2 channels per
    instruction, bf16 2x mode) + a binary add tree on [4F] chunks (bf16
    except the final fp32 fold), then ScalarE re-interleaves/upcasts to the
    fp32 [N,2] output.
"""

import ml_dtypes
import numpy as np

import concourse.bass as bass
import concourse.tile as tile
from concourse import bacc, mybir
from concourse.bass_utils import run_bass_kernel_spmd

f32 = mybir.dt.float32
bf16 = mybir.dt.bfloat16
AF = mybir.ActivationFunctionType
OP = mybir.AluOpType

# ----- problem constants (hardcoded per spec) -----
N = 2_000_000
K = 16
CH = 2
ACTIVE_DEG = 3

C0 = 0.28209479177387814
C1 = 0.4886025119029199
C2 = (1.0925484305920792, -1.0925484305920792, 0.31539156525252005,
      -1.0925484305920792, 0.5462742152960396)
C3 = (-0.5900435899266435, 2.890611442640554, -0.4570457994644658,
      0.3731763325901154, -0.4570457994644658, 1.445305721320277,
      -0.5900435899266435)

# Basis constants with the C1 hat-scaling folded in (hats carry a factor C1).
_C12 = C1 * C1
_C13 = C1 * C1 * C1
CC4 = C2[0] / _C12
CC5 = C2[1] / _C12
A6, D6 = 3.0 * C2[2] / _C12, -C2[2]
CC7 = C2[3] / _C12
CC8 = C2[4] / _C12
CC9 = C3[0] / _C13
CC10 = C3[1] / _C13
A11, D11 = 5.0 * C3[2] / _C13, -C3[2] / C1
A12, D12 = 5.0 * C3[3] / _C13, -3.0 * C3[3] / C1
A13, D13 = 5.0 * C3[4] / _C13, -C3[4] / C1
CC14 = C3[5] / _C13
CC15 = C3[6] / _C13

# ----- sharding geometry -----
NCORES = 8
PPART = 1960                 # points per partition per core
PC = 128 * PPART             # points per core = 250,880
NPAD = NCORES * PC           # 2,007,040
TF = 490                     # points per partition per tile
NT = PPART // TF             # 5 tiles


def _build_nc():
    # Inputs arrive host-preprocessed: coords as 3 planes [3, PC] fp32 and
    # sh as 32 (k,c)-planes [32, PC] bf16 — every DMA lands in the exact
    # SBUF layout compute wants, no on-chip shuffling.
    nc = bacc.Bacc("TRN2")
    coords_ext = nc.declare_dram_parameter("coords", [3, PC], f32, isOutput=False)
    sh_ext = nc.declare_dram_parameter("sh", [32, PC], bf16, isOutput=False)
    consts_ext = nc.declare_dram_parameter("consts", [128, 8], f32, isOutput=False)
    out_ext = nc.declare_dram_parameter("out", [PC, CH], f32, isOutput=True)

    coords_ap = coords_ext[:].rearrange("c (p f) -> p c f", p=128)   # [128,3,1960]
    sh_ap = sh_ext[:].rearrange("j (p f) -> p j f", p=128)           # [128,32,1960]
    out_ap = out_ext[:].rearrange("(p f) c -> p (f c)", p=128)       # [128, 3920]

    F = TF
    with tile.TileContext(nc) as tc:
        with (
            tc.tile_pool(name="pconst", bufs=1) as pconst,
            tc.tile_pool(name="psh", bufs=2) as psh,
            tc.tile_pool(name="pco", bufs=2) as pco,
            tc.tile_pool(name="psq", bufs=2) as psq,
            tc.tile_pool(name="pr", bufs=2) as pr,
            tc.tile_pool(name="ph", bufs=2) as ph,
            tc.tile_pool(name="pmono", bufs=2) as pmono,
            tc.tile_pool(name="pb", bufs=3) as pb,
            tc.tile_pool(name="pm", bufs=3) as pm,
            tc.tile_pool(name="ptree", bufs=4) as ptree,
            tc.tile_pool(name="pacc", bufs=2) as pacc,
            tc.tile_pool(name="pout", bufs=2) as pout,
            tc.tile_pool(name="pscr", bufs=2) as pscr,
        ):
            ct = pconst.tile([128, 8], f32)
            nc.sync.dma_start(out=ct[:], in_=consts_ext[:])

            for t in range(NT):
                shin = psh.tile([128, F * 32], bf16, tag="shin")
                shin3 = shin[:].rearrange("p (j f) -> p j f", f=F)
                nc.sync.dma_start(
                    out=shin3[:, 0:16, :],
                    in_=sh_ap[:, 0:16, t * F:(t + 1) * F],
                )
                nc.sync.dma_start(
                    out=shin3[:, 16:32, :],
                    in_=sh_ap[:, 16:32, t * F:(t + 1) * F],
                )
                ctile = pco.tile([128, F * 3], f32, tag="ctile")
                nc.gpsimd.dma_start(
                    out=ctile[:].rearrange("p (c f) -> p c f", f=F),
                    in_=coords_ap[:, :, t * F:(t + 1) * F],
                )

                cv = ctile[:].rearrange("p (c f) -> p c f", c=3)  # plane comps

                # d = coords - rx is precomputed on the host; square all
                # three planes in one ScalarE op
                sq = psq.tile([128, 3 * F], f32, tag="sq", bufs=1)
                nc.scalar.activation(sq[:], ctile[:], AF.Square, bias=0.0,
                                     scale=1.0)

                r2a = pr.tile([128, F], f32, tag="r2a")
                nc.vector.tensor_add(r2a[:], sq[:, 0:F], sq[:, F:2 * F])
                nc.vector.scalar_tensor_tensor(
                    r2a[:], sq[:, 2 * F:3 * F], 1e-12, r2a[:], OP.add, OP.add
                )
                inv = pr.tile([128, F], f32, tag="inv")
                nc.vector.reciprocal_approx_fast(inv[:], r2a[:])
                # sqrt(C1^2 / r2) = C1 * rsqrt(r2)
                rinv = inv
                nc.scalar.activation(rinv[:], inv[:], AF.Sqrt, bias=0.0,
                                     scale=_C12)

                # C1-scaled unit vector: d * rinvC1, rinv broadcast over the
                # three component planes in a single VectorE op
                hats = ph.tile([128, 3 * F], f32, tag="hats")
                nc.vector.tensor_tensor(
                    hats[:].rearrange("p (c f) -> p c f", c=3),
                    ctile[:].rearrange("p (c f) -> p c f", c=3),
                    rinv[:].unsqueeze(1).broadcast_to((128, 3, F)),
                    OP.mult,
                )
                X = hats[:, 0:F]
                Y = hats[:, F:2 * F]
                Z = hats[:, 2 * F:3 * F]

                sqh = ph.tile([128, 3 * F], f32, tag="sqh", bufs=1)
                nc.scalar.activation(sqh[:], hats[:], AF.Square, bias=0.0,
                                     scale=1.0)
                XX = sqh[:, 0:F]
                YY = sqh[:, F:2 * F]
                ZZ = sqh[:, 2 * F:3 * F]

                xy = pmono.tile([128, F], f32, tag="xy")
                nc.vector.tensor_mul(xy[:], X, Y)
                t8 = pmono.tile([128, F], f32, tag="t8")
                nc.vector.tensor_sub(t8[:], XX, YY)
                u9 = pmono.tile([128, F], f32, tag="u9")
                nc.vector.affine_then_add(u9[:], XX, t8[:], 2.0, 0.0)
                u15 = pmono.tile([128, F], f32, tag="u15")
                nc.vector.affine_then_add(u15[:], YY, t8[:], -2.0, 0.0)

                # ---- MAC: pair-batched products ([k2, c, f] = 4F per op) ----
                # Basis planes are packed in pairs matching consecutive k so
                # each product instruction covers 2 k's x 2 channels, and the
                # add tree runs on [4F] chunks (terms halve each level).
                def bpair_tile():
                    return pb.tile([128, 2 * F], bf16, tag="b", name="bp")

                def mk_product2(p_idx, bp):
                    m = pm.tile([128, 4 * F], bf16, tag="m", name="m")
                    in1 = shin[:, 4 * p_idx * F:(4 * p_idx + 4) * F].rearrange(
                        "p (k c f) -> p k c f", k=2, c=2)
                    in0 = bp[:].rearrange("p (k f) -> p k f", k=2) \
                        .unsqueeze(2).broadcast_to((128, 2, 2, F))
                    nc.vector.tensor_tensor(
                        m[:].rearrange("p (k c f) -> p k c f", k=2, c=2),
                        in0, in1, OP.mult)
                    return m

                def amr(out_slice, in0, in1, scale, bias):
                    scr = pscr.tile([128, 1], f32, tag="scr", name="scr")
                    nc.vector.affine_mul_reduce(out_slice, scr[:], in0, in1,
                                                scale, bias)

                def tadd(a, b_, dt):
                    tg = "treeb" if dt == bf16 else "treef"
                    nb = 5 if dt == bf16 else 2
                    o = ptree.tile([128, a.shape[1]], dt, tag=tg, name="tr",
                                   bufs=nb)
                    nc.vector.tensor_tensor(o[:], a[:], b_[:], OP.add)
                    return o

                # hb holds [C0, -Ytilde, +Ztilde, -Xtilde, +Ztilde, -Ytilde]:
                # slots 0-3 are the first two basis pairs; [2F:4F] doubles as
                # the (Z,X) hat pair and [4F:6F] as the (Z,Y) hat pair for the
                # paired deg>=2 plane multiplies.
                hb = pb.tile([128, 6 * F], bf16, tag="hb", name="hb", bufs=2)
                nc.vector.memset(hb[:, 0:F], C0)
                nc.scalar.mul(hb[:, F:2 * F], Y, -1.0)
                nc.scalar.copy(hb[:, 2 * F:3 * F], Z)
                nc.scalar.mul(hb[:, 3 * F:4 * F], X, -1.0)
                nc.scalar.copy(hb[:, 4 * F:5 * F], Z)
                nc.scalar.mul(hb[:, 5 * F:6 * F], Y, -1.0)
                Yn = hb[:, F:2 * F]
                Zb = hb[:, 2 * F:3 * F]
                ZXn = hb[:, 2 * F:4 * F]
                ZYn = hb[:, 4 * F:6 * F]
                m0 = mk_product2(0, hb[:, 0:2 * F])
                m1 = mk_product2(1, hb[:, 2 * F:4 * F])
                A = tadd(m0, m1, bf16)

                # ScalarE pre-scales one factor of each remaining plane to
                # bf16 (constants and signs folded), VectorE finishes with a
                # bf16 2x multiply against a signed hat (pair) plane.
                def spre(out_slice, pre_in, scale, bias=None):
                    if bias is None:
                        nc.scalar.mul(out_slice, pre_in, scale)
                    else:
                        nc.scalar.activation(out_slice, pre_in, AF.Identity,
                                             bias=bias, scale=scale)

                def sb_tile(w):
                    return pb.tile([128, w * F], bf16, tag="sb", name="sb",
                                   bufs=4)

                # pair (k4, k5): [c4*xy, c5*Y*Z]
                bp2 = bpair_tile()
                nc.scalar.mul(bp2[:, 0:F], xy[:], CC4)
                s5 = sb_tile(1)
                spre(s5[:], Y, CC5)
                nc.vector.tensor_mul(bp2[:, F:2 * F], s5[:], Zb)
                m2 = mk_product2(2, bp2)
                # pair (k6, k7): [a6*ZZ+d6, c7*X*Z]
                bp3 = bpair_tile()
                nc.scalar.activation(bp3[:, 0:F], ZZ, AF.Identity,
                                     bias=ct[:, 3:4], scale=A6)
                s7 = sb_tile(1)
                spre(s7[:], X, CC7)
                nc.vector.tensor_mul(bp3[:, F:2 * F], s7[:], Zb)
                m3 = mk_product2(3, bp3)
                B = tadd(m2, m3, bf16)

                # pair (k8, k9): [c8*t8, c9*u9*Y]
                bp4 = bpair_tile()
                nc.scalar.mul(bp4[:, 0:F], t8[:], CC8)
                s9 = sb_tile(1)
                spre(s9[:], u9[:], -CC9)
                nc.vector.tensor_mul(bp4[:, F:2 * F], s9[:], Yn)
                m4 = mk_product2(4, bp4)
                # pair (k10, k11): [c10*xy*Z, (a11*ZZ+d11)*Y] via (Z,Y) pair
                sp5 = sb_tile(2)
                spre(sp5[:, 0:F], xy[:], CC10)
                spre(sp5[:, F:2 * F], ZZ, -A11, bias=ct[:, 4:5])
                bp5 = bpair_tile()
                nc.vector.tensor_mul(bp5[:], sp5[:], ZYn)
                m5 = mk_product2(5, bp5)
                Cc = tadd(m4, m5, bf16)

                # pair (k12, k13): [(a12*ZZ+d12)*Z, (a13*ZZ+d13)*X] via (Z,X)
                sp6 = sb_tile(2)
                spre(sp6[:, 0:F], ZZ, A12, bias=ct[:, 5:6])
                spre(sp6[:, F:2 * F], ZZ, -A13, bias=ct[:, 6:7])
                bp6 = bpair_tile()
                nc.vector.tensor_mul(bp6[:], sp6[:], ZXn)
                m6 = mk_product2(6, bp6)
                # pair (k14, k15): [c14*t8*Z, c15*u15*X] via (Z,X) pair
                sp7 = sb_tile(2)
                spre(sp7[:, 0:F], t8[:], CC14)
                spre(sp7[:, F:2 * F], u15[:], -CC15)
                bp7 = bpair_tile()
                nc.vector.tensor_mul(bp7[:], sp7[:], ZXn)
                m7 = mk_product2(7, bp7)
                D = tadd(m6, m7, bf16)

                E = tadd(A, B, bf16)
                G = tadd(Cc, D, bf16)
                H = tadd(E, G, bf16)

                # final fold writes the interleaved fp32 output directly
                out_t = pout.tile([128, 2 * F], f32, tag="out")
                nc.vector.tensor_tensor(
                    out_t[:].rearrange("p (f c) -> p c f", c=2),
                    H[:, 0:2 * F].rearrange("p (c f) -> p c f", c=2),
                    H[:, 2 * F:4 * F].rearrange("p (c f) -> p c f", c=2),
                    OP.add,
                )
                nc.gpsimd.dma_start(
                    out=out_ap[:, t * 2 * F:(t + 1) * 2 * F], in_=out_t[:]
                )

    nc.finalize()
    return nc


_NC_CACHE = None
_last_in_maps = None


def _get_nc():
    global _NC_CACHE
    if _NC_CACHE is None:
        _NC_CACHE = _build_nc()
    return _NC_CACHE


def kernel(coordinates, active_deg, max_coeffs, sh_coefficients, rx_pos,
           **unused):
    assert int(active_deg) == ACTIVE_DEG and int(max_coeffs) == K
    coords = np.ascontiguousarray(np.asarray(coordinates, dtype=np.float32))
    sh = np.ascontiguousarray(np.asarray(sh_coefficients, dtype=np.float32))
    rx = np.asarray(rx_pos, dtype=np.float32).reshape(3)
    n = coords.shape[0]
    assert n == N and sh.shape == (N * K, CH)

    consts = np.zeros((128, 8), dtype=np.float32)
    consts[:, 0:3] = -rx[None, :]
    consts[:, 3] = D6
    consts[:, 4] = -D11
    consts[:, 5] = D12
    consts[:, 6] = -D13

    # Host-side relayout: coords -> 3 fp32 planes, sh -> 32 bf16 (k,c)-planes,
    # so device DMAs land directly in compute layout.
    sh32 = sh.reshape(n, K * CH)
    coordsT = coords.T - rx[:, None]  # [3, N], receiver offset folded in
    in_maps = []
    for c in range(NCORES):
        lo, hi = c * PC, (c + 1) * PC
        real = min(hi, n) - lo
        coords_c = np.zeros((3, PC), dtype=np.float32)
        coords_c[:, :real] = coordsT[:, lo:lo + real]
        sh_c = np.zeros((32, PC), dtype=ml_dtypes.bfloat16)
        sh_c[:, :real] = sh32[lo:lo + real].T
        in_maps.append({"coords": coords_c, "sh": sh_c, "consts": consts})

    global _last_in_maps
    _last_in_maps = in_maps
    res = run_bass_kernel_spmd(_get_nc(), in_maps, list(range(NCORES)))
    out = np.concatenate([np.asarray(res.results[c]["out"])
                          for c in range(NCORES)], axis=0)
    return out[:n]
